# revision 10
# baseline (speedup 1.0000x reference)
"""Bahdanau-attention scores kernel for one TRN2 chip (8 NeuronCores).

Reference computation (B=32, S=2048, H=1024):
    energy = tanh(hidden @ W1^T + enc @ W2^T + b)   # (B, S, H)
    scores = energy . v                             # (B, S)
    out    = softmax(scores, axis=S)[:, None, :]    # (B, 1, S)

Distribution: data-parallel over B — each of the 8 cores handles 4 batch
rows; the small tensors (attn_W, attn_b, v, hidden) are replicated.
No collectives needed; the gather is a host-side concatenation.

Per-core layout (everything pre-transposed on the host so every DMA is
contiguous):
    encT  (4, 4, 128, 8, 512) bf16  encoder pre-packed per (batch, s-chunk)
                            into the exact SBUF tile layout [p][ht][s'] so
                            every DMA is 128 contiguous 8 KB runs (descriptor
                            generation, not bandwidth, gated startup)
    w2T   (128, 8, 1024) bf16  W2^T pre-packed into SBUF layout [p][ht][k]
    hbias (128, 8, 4) f32   hidden @ W1^T + attn_b, tiled (p, kt, b) —
                            8 MFLOP of the 137 GFLOP total, folded into
                            host-side input prep
    vvf   (128, 8)   f32    v tiled (p, kt)
    ones  (128, 1)   bf16   all-ones (partition-sum stationary)
    out   (4, S)     f32

On-core dataflow (orientation: k on partitions, s on the free axis):
    eT[k, s]   = sum_h w2T[h, k] * encT[h, s]    (main matmul, PSUM f32)
    t[k, s]    = tanh(eT + hb[k, b])             (ScalarE, per-partition bias)
    acc[k, s]  = sum_kt v[k] * t[k, s]           (VectorE mul + in-place add;
                                                  last add emits bf16)
    sc[1, s]   = ones . acc                      (matmul partition-sum, emitted
                                                  one chunk late so the PE FIFO
                                                  never waits on the DVE chain)
    ex[1, s]   = exp(sc), per-chunk sums via accum_out (no max subtraction:
                 |scores| <= ||v||_1 ~ 26, exp is safe in f32 and the result
                 is mathematically identical to the max-subtracted softmax)
    out[b, s]  = ex * (1 / sum)
"""

import numpy as np

B, S, H = 32, 2048, 1024
NCORES = 8
BL = B // NCORES          # batch rows per core
P = 128                   # SBUF partitions
KT = H // P               # 8 k-tiles
HT = H // P               # 8 h-tiles
NSC = 4                   # s-chunks per row
SCW = S // NSC            # 512 (one PSUM bank of f32)

_CACHE = {}


def _build_nc():
    import concourse.bacc as bacc
    import concourse.mybir as mybir
    import concourse.tile as tile

    dt = mybir.dt
    AFT = mybir.ActivationFunctionType

    nc = bacc.Bacc("TRN2", target_bir_lowering=False, debug=False)

    encT = nc.declare_dram_parameter("encT", [BL, NSC, P, HT, SCW], dt.bfloat16, isOutput=False)
    w2T = nc.declare_dram_parameter("w2T", [P, HT, H], dt.bfloat16, isOutput=False)
    hbias = nc.declare_dram_parameter("hbias", [P, KT, BL], dt.float32, isOutput=False)
    vvf = nc.declare_dram_parameter("vvf", [P, KT], dt.float32, isOutput=False)
    ones = nc.declare_dram_parameter("ones", [P, 1], dt.bfloat16, isOutput=False)
    out_d = nc.declare_dram_parameter("out", [BL, S], dt.float32, isOutput=True)

    with tile.TileContext(nc) as tc:
        with (
            tc.tile_pool(name="const", bufs=1) as constp,
            tc.tile_pool(name="enc", bufs=3) as encp,
            tc.tile_pool(name="tanh", bufs=3) as tanhp,
            tc.tile_pool(name="accp", bufs=2) as accp,
            tc.tile_pool(name="vtp", bufs=3) as vtp,
            tc.tile_pool(name="soft", bufs=2) as softp,
            tc.tile_pool(name="pe", bufs=5, space="PSUM") as pep,
            tc.tile_pool(name="pv", bufs=2, space="PSUM") as pvp,
            tc.tile_pool(name="wu", bufs=1, space="PSUM") as wup,
        ):
            # PE warm-up: dense dummy matmuls release the HAM clock gate
            # (1.2 -> 2.4 GHz needs ~3.4us of sustained PE work) while the
            # first weight/enc DMAs are still in flight.
            wut = constp.tile([P, SCW], dt.bfloat16, tag="wut")
            nc.gpsimd.memset(wut[:], 0.0)
            wps = wup.tile([P, SCW], dt.float32)
            for _ in range(34):
                nc.tensor.matmul(wps[:], wut[:, 0:P], wut[:], start=True, stop=True)

            # small constants first on the scalar ring (16 KB), then the
            # stationary weights split across BOTH HWDGE rings so the 2 MB
            # of w2T and the first enc chunk stream at aggregate HBM BW
            hb = constp.tile([P, KT, BL], dt.float32)
            nc.scalar.dma_start(hb[:], hbias.ap())
            vvs = constp.tile([P, KT], dt.float32)
            nc.scalar.dma_start(vvs[:], vvf.ap())
            on1 = constp.tile([P, 1], dt.bfloat16)
            nc.scalar.dma_start(on1[:], ones.ap())
            w2s = constp.tile([P, HT, H], dt.bfloat16)
            nc.sync.dma_start(w2s[:, 0:HT // 2, :], w2T[:, 0:HT // 2, :])
            nc.sync.dma_start(w2s[:, HT // 2:, :], w2T[:, HT // 2:, :])

            ex_tiles = {}
            sm_tiles = {}
            pending = None

            def finish_chunk(p):
                # partition-sum + online exp for an already-computed chunk;
                # called one chunk later so the PE never waits on DVE
                pb, psc, pacc_bf = p
                pv = pvp.tile([1, SCW], dt.float32)
                nc.tensor.matmul(pv[:], on1[:], pacc_bf[:], start=True, stop=True)
                nc.scalar.activation(
                    ex_tiles[pb][:, psc * SCW:(psc + 1) * SCW], pv[:], AFT.Exp,
                    accum_out=sm_tiles[pb][:, psc:psc + 1],
                )

            def finish_b(pb):
                ssum = softp.tile([1, 1], dt.float32, tag="ssum")
                nc.vector.tensor_reduce(
                    ssum[:], sm_tiles[pb][:], axis=mybir.AxisListType.X,
                    op=mybir.AluOpType.add,
                )
                rc = softp.tile([1, 1], dt.float32, tag="rc")
                nc.vector.reciprocal(rc[:], ssum[:])
                ot = softp.tile([1, S], dt.float32, tag="ot")
                nc.vector.tensor_scalar_mul(ot[:], ex_tiles[pb][:], rc[:])
                nc.sync.dma_start(out_d[pb:pb + 1, :], ot[:])

            for b in range(BL):
                ex_tiles[b] = softp.tile([1, S], dt.float32, tag="ex", name="ex")
                sm_tiles[b] = softp.tile([1, NSC], dt.float32, tag="sm4", name="sm4")
                for sc in range(NSC):
                    et = encp.tile([P, HT, SCW], dt.bfloat16)
                    dma_eng = nc.sync if (b == 0 and sc == 0) else nc.gpsimd
                    enc_dma = dma_eng.dma_start(et[:], encT[b][sc])
                    if b == 0 and sc == 0:
                        first_enc_dma = enc_dma
                    elif b == 0 and sc <= 2:
                        # keep startup HBM bandwidth for the critical path:
                        # prefetches wait until the first chunk has landed
                        tile.add_dep_helper(
                            enc_dma.ins, first_enc_dma.ins,
                            reason="prefetch yields startup bandwidth",
                        )
                    acc = accp.tile([P, SCW], dt.float32)
                    acc_bf = vtp.tile([P, SCW], dt.bfloat16, tag="accbf")
                    for kt in range(KT):
                        pe = pep.tile([P, SCW], dt.float32)
                        for ht in range(HT):
                            nc.tensor.matmul(
                                pe[:],
                                w2s[:, ht, kt * P:(kt + 1) * P],
                                et[:, ht, :],
                                start=(ht == 0),
                                stop=(ht == HT - 1),
                            )
                        th = tanhp.tile([P, SCW], dt.bfloat16)
                        nc.scalar.activation(
                            th[:], pe[:], AFT.Tanh, bias=hb[:, kt, b:b + 1]
                        )
                        if kt == 0:
                            nc.vector.tensor_scalar_mul(acc[:], th[:], vvs[:, 0:1])
                        else:
                            vt = vtp.tile([P, SCW], dt.float32, tag="vt")
                            nc.vector.tensor_scalar_mul(vt[:], th[:], vvs[:, kt:kt + 1])
                            dst = acc_bf if kt == KT - 1 else acc
                            nc.vector.tensor_add(dst[:], acc[:], vt[:])
                    if pending is not None:
                        finish_chunk(pending)
                        if pending[1] == NSC - 1:
                            finish_b(pending[0])
                    pending = (b, sc, acc_bf)
            finish_chunk(pending)
            finish_b(pending[0])

    nc.compile()
    return nc


def _get_nc():
    if "nc" not in _CACHE:
        _CACHE["nc"] = _build_nc()
    return _CACHE["nc"]


def _make_in_maps(hidden, encoder_outputs, attn_W, attn_b, v):
    import concourse.mybir as mybir

    bf16 = mybir.dt.np(mybir.dt.bfloat16)
    f32 = np.float32

    # [p][ht][k] pack of W2^T
    w2T = np.ascontiguousarray(
        attn_W[:, H:].T.reshape(HT, P, H).transpose(1, 0, 2)
    ).astype(bf16)
    vvt = np.ascontiguousarray(v.reshape(KT, P).T).astype(f32)
    ones = np.ones((P, 1), dtype=bf16)
    hid = hidden[0]  # (B, H)
    # hidden-term: (B, H) @ (H, H)^T + b — 8 MFLOP, f32-exact on host
    hterm = (hid @ attn_W[:, :H].T + attn_b).astype(f32)  # (B, H)

    in_maps = []
    for c in range(NCORES):
        sl = slice(c * BL, (c + 1) * BL)
        # [b][sc][p][ht][s'] pack: encT[b,sc,p,t,s'] = enc[b, sc*512+s', t*128+p]
        encT = np.ascontiguousarray(
            encoder_outputs[sl]
            .reshape(BL, NSC, SCW, HT, P)
            .transpose(0, 1, 4, 3, 2)
        ).astype(bf16)
        # hbias[p, kt, b] = hterm[b, kt*128 + p]
        hbias = np.ascontiguousarray(hterm[sl].T.reshape(KT, P, BL).transpose(1, 0, 2))
        in_maps.append(
            {
                "encT": encT,
                "w2T": w2T,
                "hbias": hbias,
                "vvf": vvt,
                "ones": ones,
            }
        )
    return in_maps


def kernel(hidden, encoder_outputs, attn_W, attn_b, v):
    from concourse.bass_utils import run_bass_kernel_spmd

    nc = _get_nc()
    in_maps = _make_in_maps(
        np.asarray(hidden, dtype=np.float32),
        np.asarray(encoder_outputs, dtype=np.float32),
        np.asarray(attn_W, dtype=np.float32),
        np.asarray(attn_b, dtype=np.float32),
        np.asarray(v, dtype=np.float32),
    )
    # A freshly-opened device occasionally fails its first execution with
    # NRT_EXEC_UNIT_UNRECOVERABLE; a retry on the reset device succeeds.
    last_err = None
    for attempt in range(3):
        try:
            res = run_bass_kernel_spmd(nc, in_maps, core_ids=list(range(NCORES)))
            break
        except Exception as e:
            last_err = e
            import time
            time.sleep(2.0)
    else:
        raise last_err
    out = np.concatenate([res.results[c]["out"] for c in range(NCORES)], axis=0)
    return out[:, None, :].astype(np.float32)


# revision 11
# speedup vs baseline: 1.0172x; 1.0172x over previous
"""Bahdanau-attention scores kernel for one TRN2 chip (8 NeuronCores).

Reference computation (B=32, S=2048, H=1024):
    energy = tanh(hidden @ W1^T + enc @ W2^T + b)   # (B, S, H)
    scores = energy . v                             # (B, S)
    out    = softmax(scores, axis=S)[:, None, :]    # (B, 1, S)

Distribution: data-parallel over B — each of the 8 cores handles 4 batch
rows; the small tensors (attn_W, attn_b, v, hidden) are replicated.
No collectives needed; the gather is a host-side concatenation.

Per-core layout (everything pre-transposed on the host so every DMA is
contiguous):
    encT  (4, 4, 128, 8, 512) bf16  encoder pre-packed per (batch, s-chunk)
                            into the exact SBUF tile layout [p][ht][s'] so
                            every DMA is 128 contiguous 8 KB runs (descriptor
                            generation, not bandwidth, gated startup)
    w2T   (128, 8, 1024) bf16  W2^T pre-packed into SBUF layout [p][ht][k]
    hbias (128, 8, 4) f32   hidden @ W1^T + attn_b, tiled (p, kt, b) —
                            8 MFLOP of the 137 GFLOP total, folded into
                            host-side input prep
    vvf   (128, 8)   f32    v tiled (p, kt)
    ones  (128, 1)   bf16   all-ones (partition-sum stationary)
    out   (4, S)     f32

On-core dataflow (orientation: k on partitions, s on the free axis):
    eT[k, s]   = sum_h w2T[h, k] * encT[h, s]    (main matmul, PSUM f32)
    t[k, s]    = tanh(eT + hb[k, b])             (ScalarE, per-partition bias)
    acc[k, s]  = sum_kt v[k] * t[k, s]           (VectorE mul + in-place add;
                                                  last add emits bf16)
    sc[1, s]   = ones . acc                      (matmul partition-sum, emitted
                                                  one chunk late so the PE FIFO
                                                  never waits on the DVE chain)
    ex[1, s]   = exp(sc), per-chunk sums via accum_out (no max subtraction:
                 |scores| <= ||v||_1 ~ 26, exp is safe in f32 and the result
                 is mathematically identical to the max-subtracted softmax)
    out[b, s]  = ex * (1 / sum)
"""

import numpy as np

B, S, H = 32, 2048, 1024
NCORES = 8
BL = B // NCORES          # batch rows per core
P = 128                   # SBUF partitions
KT = H // P               # 8 k-tiles
HT = H // P               # 8 h-tiles
NSC = 4                   # s-chunks per row
SCW = S // NSC            # 512 (one PSUM bank of f32)

_CACHE = {}


def _build_nc():
    import concourse.bacc as bacc
    import concourse.mybir as mybir
    import concourse.tile as tile

    dt = mybir.dt
    AFT = mybir.ActivationFunctionType

    nc = bacc.Bacc("TRN2", target_bir_lowering=False, debug=False)

    encT = nc.declare_dram_parameter("encT", [BL, NSC, P, HT, SCW], dt.bfloat16, isOutput=False)
    w2T = nc.declare_dram_parameter("w2T", [P, HT, H], dt.bfloat16, isOutput=False)
    hbias = nc.declare_dram_parameter("hbias", [P, KT, BL], dt.float32, isOutput=False)
    vvf = nc.declare_dram_parameter("vvf", [P, KT], dt.float32, isOutput=False)
    ones = nc.declare_dram_parameter("ones", [P, 1], dt.bfloat16, isOutput=False)
    out_d = nc.declare_dram_parameter("out", [BL, S], dt.float32, isOutput=True)

    with tile.TileContext(nc) as tc:
        with (
            tc.tile_pool(name="const", bufs=1) as constp,
            tc.tile_pool(name="enc", bufs=3) as encp,
            tc.tile_pool(name="tanh", bufs=3) as tanhp,
            tc.tile_pool(name="accp", bufs=2) as accp,
            tc.tile_pool(name="vtp", bufs=3) as vtp,
            tc.tile_pool(name="soft", bufs=2) as softp,
            tc.tile_pool(name="pe", bufs=5, space="PSUM") as pep,
            tc.tile_pool(name="pv", bufs=2, space="PSUM") as pvp,
            tc.tile_pool(name="wu", bufs=1, space="PSUM") as wup,
        ):
            # PE warm-up: dense dummy matmuls release the HAM clock gate
            # (1.2 -> 2.4 GHz needs ~3.4us of sustained PE work) while the
            # first weight/enc DMAs are still in flight.
            wut = constp.tile([P, SCW], dt.bfloat16, tag="wut")
            nc.gpsimd.memset(wut[:], 0.0)
            wps = wup.tile([P, SCW], dt.float32)
            for _ in range(20):
                nc.tensor.matmul(wps[:], wut[:, 0:P], wut[:], start=True, stop=True)

            # small constants first on the scalar ring (16 KB), then the
            # stationary weights split across BOTH HWDGE rings so the 2 MB
            # of w2T and the first enc chunk stream at aggregate HBM BW
            hb = constp.tile([P, KT, BL], dt.float32)
            nc.scalar.dma_start(hb[:], hbias.ap())
            vvs = constp.tile([P, KT], dt.float32)
            nc.scalar.dma_start(vvs[:], vvf.ap())
            on1 = constp.tile([P, 1], dt.bfloat16)
            nc.scalar.dma_start(on1[:], ones.ap())
            # w2T and the first enc chunk interleave on the sync ring at
            # half-tile granularity (separate tiles, so deps are per-half):
            # the PE starts after w2s_a + et0_a (2 MB) instead of 3 MB
            w2s_a = constp.tile([P, HT // 2, H], dt.bfloat16)
            w2s_b = constp.tile([P, HT // 2, H], dt.bfloat16)

            ex_tiles = {}
            sm_tiles = {}
            pending = None

            def finish_chunk(p):
                # partition-sum + online exp for an already-computed chunk;
                # called one chunk later so the PE never waits on DVE
                pb, psc, pacc_bf = p
                pv = pvp.tile([1, SCW], dt.float32)
                nc.tensor.matmul(pv[:], on1[:], pacc_bf[:], start=True, stop=True)
                nc.scalar.activation(
                    ex_tiles[pb][:, psc * SCW:(psc + 1) * SCW], pv[:], AFT.Exp,
                    accum_out=sm_tiles[pb][:, psc:psc + 1],
                )

            def finish_b(pb):
                ssum = softp.tile([1, 1], dt.float32, tag="ssum")
                nc.vector.tensor_reduce(
                    ssum[:], sm_tiles[pb][:], axis=mybir.AxisListType.X,
                    op=mybir.AluOpType.add,
                )
                rc = softp.tile([1, 1], dt.float32, tag="rc")
                nc.vector.reciprocal(rc[:], ssum[:])
                ot = softp.tile([1, S], dt.float32, tag="ot")
                nc.vector.tensor_scalar_mul(ot[:], ex_tiles[pb][:], rc[:])
                nc.sync.dma_start(out_d[pb:pb + 1, :], ot[:])

            for b in range(BL):
                ex_tiles[b] = softp.tile([1, S], dt.float32, tag="ex", name="ex")
                sm_tiles[b] = softp.tile([1, NSC], dt.float32, tag="sm4", name="sm4")
                for sc in range(NSC):
                    et_a = encp.tile([P, HT // 2, SCW], dt.bfloat16, tag="eta")
                    et_b = encp.tile([P, HT // 2, SCW], dt.bfloat16, tag="etb")
                    if b == 0 and sc == 0:
                        nc.sync.dma_start(w2s_a[:], w2T[:, 0:HT // 2, :])
                        nc.sync.dma_start(et_a[:], encT[0][0][:, 0:HT // 2, :])
                        nc.sync.dma_start(w2s_b[:], w2T[:, HT // 2:, :])
                        first_enc_dma = nc.sync.dma_start(
                            et_b[:], encT[0][0][:, HT // 2:, :]
                        )
                    else:
                        d1 = nc.gpsimd.dma_start(et_a[:], encT[b][sc][:, 0:HT // 2, :])
                        d2 = nc.gpsimd.dma_start(et_b[:], encT[b][sc][:, HT // 2:, :])
                        if b == 0 and sc <= 2:
                            # keep startup HBM bandwidth for the critical path:
                            # prefetches wait until the first chunk has landed
                            for d in (d1, d2):
                                tile.add_dep_helper(
                                    d.ins, first_enc_dma.ins,
                                    reason="prefetch yields startup bandwidth",
                                )
                    acc = accp.tile([P, SCW], dt.float32)
                    acc_bf = vtp.tile([P, SCW], dt.bfloat16, tag="accbf")
                    for kt in range(KT):
                        pe = pep.tile([P, SCW], dt.float32)
                        for ht in range(HT):
                            w2h = w2s_a if ht < HT // 2 else w2s_b
                            eth = et_a if ht < HT // 2 else et_b
                            nc.tensor.matmul(
                                pe[:],
                                w2h[:, ht % (HT // 2), kt * P:(kt + 1) * P],
                                eth[:, ht % (HT // 2), :],
                                start=(ht == 0),
                                stop=(ht == HT - 1),
                            )
                        th = tanhp.tile([P, SCW], dt.bfloat16)
                        nc.scalar.activation(
                            th[:], pe[:], AFT.Tanh, bias=hb[:, kt, b:b + 1]
                        )
                        if kt == 0:
                            nc.vector.tensor_scalar_mul(acc[:], th[:], vvs[:, 0:1])
                        else:
                            vt = vtp.tile([P, SCW], dt.float32, tag="vt")
                            nc.vector.tensor_scalar_mul(vt[:], th[:], vvs[:, kt:kt + 1])
                            dst = acc_bf if kt == KT - 1 else acc
                            nc.vector.tensor_add(dst[:], acc[:], vt[:])
                    if pending is not None:
                        finish_chunk(pending)
                        if pending[1] == NSC - 1:
                            finish_b(pending[0])
                    pending = (b, sc, acc_bf)
            finish_chunk(pending)
            finish_b(pending[0])

    nc.compile()
    return nc


def _get_nc():
    if "nc" not in _CACHE:
        _CACHE["nc"] = _build_nc()
    return _CACHE["nc"]


def _make_in_maps(hidden, encoder_outputs, attn_W, attn_b, v):
    import concourse.mybir as mybir

    bf16 = mybir.dt.np(mybir.dt.bfloat16)
    f32 = np.float32

    # [p][ht][k] pack of W2^T
    w2T = np.ascontiguousarray(
        attn_W[:, H:].T.reshape(HT, P, H).transpose(1, 0, 2)
    ).astype(bf16)
    vvt = np.ascontiguousarray(v.reshape(KT, P).T).astype(f32)
    ones = np.ones((P, 1), dtype=bf16)
    hid = hidden[0]  # (B, H)
    # hidden-term: (B, H) @ (H, H)^T + b — 8 MFLOP, f32-exact on host
    hterm = (hid @ attn_W[:, :H].T + attn_b).astype(f32)  # (B, H)

    in_maps = []
    for c in range(NCORES):
        sl = slice(c * BL, (c + 1) * BL)
        # [b][sc][p][ht][s'] pack: encT[b,sc,p,t,s'] = enc[b, sc*512+s', t*128+p]
        encT = np.ascontiguousarray(
            encoder_outputs[sl]
            .reshape(BL, NSC, SCW, HT, P)
            .transpose(0, 1, 4, 3, 2)
        ).astype(bf16)
        # hbias[p, kt, b] = hterm[b, kt*128 + p]
        hbias = np.ascontiguousarray(hterm[sl].T.reshape(KT, P, BL).transpose(1, 0, 2))
        in_maps.append(
            {
                "encT": encT,
                "w2T": w2T,
                "hbias": hbias,
                "vvf": vvt,
                "ones": ones,
            }
        )
    return in_maps


def kernel(hidden, encoder_outputs, attn_W, attn_b, v):
    from concourse.bass_utils import run_bass_kernel_spmd

    nc = _get_nc()
    in_maps = _make_in_maps(
        np.asarray(hidden, dtype=np.float32),
        np.asarray(encoder_outputs, dtype=np.float32),
        np.asarray(attn_W, dtype=np.float32),
        np.asarray(attn_b, dtype=np.float32),
        np.asarray(v, dtype=np.float32),
    )
    # A freshly-opened device occasionally fails its first execution with
    # NRT_EXEC_UNIT_UNRECOVERABLE; a retry on the reset device succeeds.
    last_err = None
    for attempt in range(3):
        try:
            res = run_bass_kernel_spmd(nc, in_maps, core_ids=list(range(NCORES)))
            break
        except Exception as e:
            last_err = e
            import time
            time.sleep(2.0)
    else:
        raise last_err
    out = np.concatenate([res.results[c]["out"] for c in range(NCORES)], axis=0)
    return out[:, None, :].astype(np.float32)
